# revision 2
# baseline (speedup 1.0000x reference)
"""MoE expert-parallel kernel for Trainium2 (8 NeuronCores), v2.

Problem: nn_Experts (T=8192 tokens, d_model=1024, d_ff=4096, E=8 experts,
top-k=2).  out[t] = sum_e w[t,e] * (relu(x[t] @ wi[e].T) @ wo[e].T), where
w[t,e] is the combined routing weight (0 for unrouted pairs).

Strategy (v2 = expert-pair / d_ff-split parallelism):
  - Experts are ranked by routed-token count; the top-4 become "A slots",
    the bottom-4 "B slots" (shared capacities CA = pad8(max A count),
    CB = pad8(max B count)).  Pair p = (A[p], B[p]) is assigned to cores
    {2p, 2p+1}: each core holds HALF the d_ff rows of wi/wo for both
    experts of its pair (same SBUF budget as one full expert) and
    processes ALL tokens routed to either expert over that half.  The two
    halves' partial outputs are summed on the host.  Per-core effective
    load drops from pad8(max count) to (CA+CB)/2 and the program stays
    SPMD (identical shapes on every core).
  - Device (per core): chunked dense fused MLP, software-pipelined:
    mm1 of chunk i+1 is emitted before mm2 of chunk i so the PE never
    waits on the relu (scalar engine) at chunk boundaries.  Weights are
    bf16 and SBUF-resident; matmuls accumulate in fp32 PSUM; the partial
    y output is stored bf16 (halves DMA; host sums the two halves and
    applies routing weights in fp32).
  - Host: routing, gather/pack (per pair, shared by its 2 cores),
    scatter-add combine.

Measured numerics (vs fp32 reference): max-abs rel err ~5e-3.
"""
import os
import sys
from contextlib import ExitStack

import numpy as np

sys.path.insert(0, "/opt/trn_rl_repo")

import concourse.bass as bass
import concourse.mybir as mybir
from concourse import tile
from concourse import bass2jax
from concourse.bass2jax import _bass_exec_p, install_neuronx_cc_hook

T, D_MODEL, D_FF, N_EXPERTS, TOP_K = 8192, 1024, 4096, 8, 2
N_CORES = 8
N_PAIRS = 4
FH = D_FF // 2       # d_ff rows per core (f-half)
P = 128              # partitions
TC = 512             # max token chunk (one PSUM bank of fp32)
KD = D_MODEL // P    # 8 contraction steps for mm1 / output tiles for mm2
MFH = FH // P        # 16 ff tiles per half
COMPUTE_DT = mybir.dt.bfloat16


def split_multi_waits(nc, max_waits=1):
    """This container's walrus codegen rejects instructions carrying more
    than a couple of semaphore waits (e.g. the TileContext tail Drain).
    Move excess waits onto preceding NoOps on the same engine."""
    for f in nc.m.functions:
        for b in f.blocks:
            il = b.instructions
            i = 0
            while i < len(il):
                inst = il[i]
                si = inst.sync_info
                if si is not None and len(si.on_wait) > max_waits:
                    waits = list(si.on_wait)
                    si.on_wait = waits[:max_waits]
                    inst.sync_info = si
                    pre = []
                    rest = waits[max_waits:]
                    for k in range(0, len(rest), max_waits):
                        nop = mybir.InstNoOp(
                            name=f"{inst.name}-ws-{k}", ins=[], outs=[])
                        nop.engine = inst.engine
                        nop.sync_info = mybir.SyncInfo(
                            on_wait=rest[k:k + max_waits], on_update=[])
                        pre.append(nop)
                    for n in reversed(pre):
                        il.insert(i, n)
                    i += len(pre)
                i += 1


class SpmdRunner:
    """Compile a Bass program once; run it SPMD on n_cores via PJRT/axon."""

    def __init__(self, nc, n_cores):
        import jax
        from jax.sharding import Mesh, PartitionSpec
        from jax.experimental.shard_map import shard_map

        install_neuronx_cc_hook()
        self.nc = nc
        self.n_cores = n_cores
        partition_name = (nc.partition_id_tensor.name
                          if nc.partition_id_tensor else None)
        in_names, out_names, out_avals, zero_outs = [], [], [], []
        for alloc in nc.m.functions[0].allocations:
            if not isinstance(alloc, mybir.MemoryLocationSet):
                continue
            name = alloc.memorylocations[0].name
            if alloc.kind == "ExternalInput":
                if name != partition_name:
                    in_names.append(name)
            elif alloc.kind == "ExternalOutput":
                out_names.append(name)
                shape = tuple(alloc.tensor_shape)
                dtype = mybir.dt.np(alloc.dtype)
                out_avals.append(jax.core.ShapedArray(shape, dtype))
                zero_outs.append(np.zeros(shape, dtype))
        self.in_names = in_names
        self.out_names = out_names
        self.out_avals = out_avals
        self.zero_outs = zero_outs
        n_params = len(in_names)
        n_outs = len(out_avals)
        all_in_names = list(in_names) + list(out_names)
        if partition_name is not None:
            all_in_names.append(partition_name)
        donate = tuple(range(n_params, n_params + n_outs))

        def _body(*args):
            operands = list(args)
            if partition_name is not None:
                operands.append(bass2jax.partition_id_tensor())
            outs = _bass_exec_p.bind(
                *operands,
                out_avals=tuple(out_avals),
                in_names=tuple(all_in_names),
                out_names=tuple(out_names),
                lowering_input_output_aliases=(),
                sim_require_finite=True,
                sim_require_nnan=True,
                nc=nc,
            )
            return tuple(outs)

        devices = jax.devices()[:n_cores]
        assert len(devices) == n_cores, (
            f"need {n_cores} neuron cores, found {len(jax.devices())}")
        mesh = Mesh(np.asarray(devices), ("core",))
        self.mesh = mesh
        in_specs = (PartitionSpec("core"),) * (n_params + n_outs)
        out_specs = (PartitionSpec("core"),) * n_outs
        self.sharded = jax.jit(
            shard_map(_body, mesh=mesh, in_specs=in_specs,
                      out_specs=out_specs, check_rep=False),
            donate_argnums=donate, keep_unused=True)

    def prep(self, in_maps):
        n = self.n_cores
        concat_in = [
            np.concatenate([np.asarray(in_maps[c][name]) for c in range(n)],
                           axis=0)
            for name in self.in_names
        ]
        concat_zeros = self.device_zeros()
        return concat_in, concat_zeros

    def device_zeros(self):
        """Donated output buffers, created directly on device (no H2D)."""
        import jax
        import jax.numpy as jnp
        from jax.sharding import NamedSharding, PartitionSpec
        if not hasattr(self, "_zeros_fn"):
            n = self.n_cores
            shapes = [(n * z.shape[0], *z.shape[1:]) for z in self.zero_outs]
            dts = [z.dtype for z in self.zero_outs]
            sh = tuple(NamedSharding(self.mesh, PartitionSpec("core"))
                       for _ in shapes)
            self._zeros_fn = jax.jit(
                lambda: tuple(jnp.zeros(s, d) for s, d in zip(shapes, dts)),
                out_shardings=sh)
        return list(self._zeros_fn())

    def run_prepped(self, concat_in, concat_zeros=None):
        if concat_zeros is None:
            concat_zeros = self.device_zeros()
        return self.sharded(*concat_in, *concat_zeros)

    def __call__(self, in_maps):
        out_arrs = self.run_prepped(*self.prep(in_maps))
        n = self.n_cores
        return [
            {name: np.asarray(out_arrs[i]).reshape(
                n, *self.out_avals[i].shape)[c]
             for i, name in enumerate(self.out_names)}
            for c in range(n)
        ]


def chunk_widths(C):
    """Split C tokens into near-equal chunks of width <= TC (multiple of 8),
    to avoid padding the capacity all the way up to a TC multiple."""
    n = -(-C // TC)
    w = -(-(-(-C // n)) // 8) * 8
    widths = [w] * (n - 1) + [C - w * (n - 1)]
    assert all(0 < x <= TC for x in widths) and sum(widths) == C, (C, widths)
    return widths


def seg_chunks(CA, CB):
    """Chunk list [(seg, col0, width), ...]: A-segment chunks then B."""
    out = []
    c0 = 0
    for seg, C in ((0, CA), (1, CB)):
        for w in chunk_widths(C):
            out.append((seg, c0, w))
            c0 += w
    return out


def build_nc(CA, CB, n_repeat=1):
    """Per-core fused pair/half MLP.

    Inputs (per core): xT [nchunk*D_MODEL, TCW] bf16 packed chunk-major
    (A-expert token chunks then B-expert), wiT [D_MODEL, D_FF] bf16 =
    [wi[eA].T half | wi[eB].T half], woT [D_FF, D_MODEL] bf16 =
    [wo[eA].T half rows ; wo[eB].T half rows].
    Output: yT [D_MODEL, CA+CB] bf16 (partial: this core's f-half).
    n_repeat>1 wraps the sweep in a hardware loop (for slope timing)."""
    chunks = seg_chunks(CA, CB)
    TCW = max(w for _, _, w in chunks)
    nchunk = len(chunks)
    C = CA + CB
    nc = bass.Bass()
    # x is packed chunk-major by the host: chunk c occupies rows
    # [c*D_MODEL, (c+1)*D_MODEL), columns [0, widths[c]) — every per-tile
    # DMA is then a contiguous block instead of C-strided lines.
    xT = nc.declare_dram_parameter("xT", [nchunk * D_MODEL, TCW],
                                   COMPUTE_DT, isOutput=False)
    wiT = nc.declare_dram_parameter("wiT", [D_MODEL, D_FF], COMPUTE_DT,
                                    isOutput=False)
    woT = nc.declare_dram_parameter("woT", [D_FF, D_MODEL], COMPUTE_DT,
                                    isOutput=False)
    yT = nc.declare_dram_parameter("yT", [D_MODEL, C], COMPUTE_DT,
                                   isOutput=True)

    with ExitStack() as ctx:
        tc = ctx.enter_context(tile.TileContext(nc))
        wpool = ctx.enter_context(tc.tile_pool(name="w", bufs=1))
        xpool = ctx.enter_context(tc.tile_pool(name="x", bufs=3))
        hpool = ctx.enter_context(tc.tile_pool(name="h", bufs=1))
        ypool = ctx.enter_context(tc.tile_pool(name="y", bufs=4))
        pspool = ctx.enter_context(
            tc.tile_pool(name="ps", bufs=8, space="PSUM"))

        def x_dma(t, ci, k, w):
            r0 = ci * D_MODEL + k * P
            nc.sync.dma_start(out=t[:], in_=xT[r0:r0 + P, 0:w])

        # x for chunk 0 first so the first matmuls start after ~1.3 MB of
        # DMA; then wi in quarters, then wo.
        x0_t = []
        for k in range(KD):
            t = xpool.tile([P, chunks[0][2]], COMPUTE_DT, tag=f"x{k}")
            x_dma(t, 0, k, chunks[0][2])
            x0_t.append(t)
        NQ = 4
        QF = D_FF // NQ
        wi_t = [[None] * NQ for _ in range(KD)]
        for q in range(NQ):
            for k in range(KD):
                t = wpool.tile([P, QF], COMPUTE_DT, tag=f"wi{k}_{q}")
                nc.sync.dma_start(
                    out=t[:], in_=wiT[k * P:(k + 1) * P,
                                      q * QF:(q + 1) * QF])
                wi_t[k][q] = t
        wo_t = []
        for m in range(2 * MFH):
            t = wpool.tile([P, D_MODEL], COMPUTE_DT, tag=f"wo{m}")
            nc.sync.dma_start(out=t[:], in_=woT[m * P:(m + 1) * P, :])
            wo_t.append(t)

        def mm1(ci, x_t, h_t):
            """First layer + relu for chunk ci; returns h tiles (bf16)."""
            seg, _, w = chunks[ci]
            for m in range(MFH):
                gm = seg * MFH + m          # global ff-tile index
                q, mq = divmod(gm, QF // P)
                ps = pspool.tile([P, w], mybir.dt.float32, tag="ps",
                                 name="ps1")
                for k in range(KD):
                    nc.tensor.matmul(
                        ps[:], wi_t[k][q][:, mq * P:(mq + 1) * P],
                        x_t[k][:, 0:w], start=k == 0, stop=k == KD - 1)
                h = hpool.tile([P, w], COMPUTE_DT, tag=f"h{m}_{ci % 2}",
                               name="h")
                nc.scalar.activation(h[:], ps[:],
                                     mybir.ActivationFunctionType.Relu)
                h_t.append(h)

        def mm2(ci, h_t):
            seg, c0, w = chunks[ci]
            for n in range(KD):
                ps = pspool.tile([P, w], mybir.dt.float32, tag="ps",
                                 name="ps2")
                for m in range(MFH):
                    nc.tensor.matmul(
                        ps[:], wo_t[seg * MFH + m][:, n * P:(n + 1) * P],
                        h_t[m][:], start=m == 0, stop=m == MFH - 1)
                y = ypool.tile([P, w], COMPUTE_DT, tag="y", name="y")
                nc.vector.tensor_copy(y[:], ps[:])
                nc.sync.dma_start(out=yT[n * P:(n + 1) * P, c0:c0 + w],
                                  in_=y[:])

        def x_tiles(ci, first):
            if first and ci == 0:
                return x0_t
            _, _, w = chunks[ci]
            ts = []
            for k in range(KD):
                t = xpool.tile([P, w], COMPUTE_DT, tag=f"x{k}", name="xt")
                x_dma(t, ci, k, w)
                ts.append(t)
            return ts

        def sweep(first=False):
            # software pipeline: mm1(i+1) is emitted before mm2(i) so the
            # PE never stalls on the relu at a chunk boundary.
            h_cur = []
            mm1(0, x_tiles(0, first), h_cur)
            for ci in range(1, nchunk):
                h_next = []
                mm1(ci, x_tiles(ci, first), h_next)
                mm2(ci - 1, h_cur)
                h_cur = h_next
            mm2(nchunk - 1, h_cur)

        if n_repeat == 1:
            sweep(first=True)
        else:
            with tc.For_i(0, n_repeat, 1,
                          hint_engines=(mybir.EngineType.PE,)):
                sweep()

    split_multi_waits(nc)
    return nc


_RUNNERS = {}


def _get_runner(CA, CB, n_repeat=1):
    key = (CA, CB, n_repeat)
    if key not in _RUNNERS:
        _RUNNERS[key] = SpmdRunner(build_nc(CA, CB, n_repeat), N_CORES)
    return _RUNNERS[key]


def _route(hidden_states, selected_experts, routing_weights):
    """Combined per-token weight for each expert, per-expert token lists,
    and the A/B slot assignment + pairing."""
    mask = selected_experts.astype(np.float32)          # [T, K, E]
    w_te = np.einsum('tke,tk->te', mask, routing_weights.astype(np.float32))
    idx = [np.nonzero(w_te[:, e] > 0)[0] for e in range(N_EXPERTS)]
    counts = np.array([len(i) for i in idx])
    order = np.argsort(-counts, kind="stable")
    slot_a = [int(e) for e in order[:N_PAIRS]]
    slot_b = [int(e) for e in order[N_EXPERTS - 1:N_PAIRS - 1:-1]]
    CA = max(8, -(-int(counts[slot_a].max()) // 8) * 8)
    CB = max(8, -(-int(counts[slot_b].max()) // 8) * 8)
    return w_te, idx, slot_a, slot_b, CA, CB


def to_bf16(a):
    """Vectorized fp32 -> bf16 cast (round-to-nearest-even), ~3x faster
    than ml_dtypes astype.  Matches ml_dtypes/hardware rounding for finite
    values (inputs here are well-scaled gaussians)."""
    import ml_dtypes
    a = np.ascontiguousarray(a, dtype=np.float32)
    u = a.view(np.uint32)
    r = ((u + 0x7FFF + ((u >> 16) & 1)) >> 16).astype(np.uint16)
    return r.view(ml_dtypes.bfloat16).reshape(a.shape)


def pack_x(hidden_states, ia, ib, CA, CB):
    """Per-pair x, chunk-major: [nchunk*D_MODEL, TCW] bf16 so every
    in-kernel x DMA is a contiguous block."""
    import ml_dtypes
    chunks = seg_chunks(CA, CB)
    TCW = max(w for _, _, w in chunks)
    xa = to_bf16(hidden_states[ia].transpose(1, 0))      # [D_MODEL, na]
    xb = to_bf16(hidden_states[ib].transpose(1, 0))
    out = np.zeros((len(chunks) * D_MODEL, TCW), dtype=ml_dtypes.bfloat16)
    off = {0: 0, 1: 0}
    src = {0: xa, 1: xb}
    for c, (seg, _, w) in enumerate(chunks):
        xg = src[seg]
        o = off[seg]
        piece = xg[:, o:min(o + w, xg.shape[1])]
        out[c * D_MODEL:(c + 1) * D_MODEL, :piece.shape[1]] = piece
        off[seg] = o + w
    return out


# Cached device-resident weight uploads.
_WEIGHT_CACHE = []


def _pack_weights(wi, wo, slot_a, slot_b, key_shape):
    """bf16-pack and upload the per-core pair/half transposed weights once;
    reuse the device arrays on later calls with identical weights."""
    import jax
    for cwi, cwo, ckey, dev in _WEIGHT_CACHE:
        if ckey == key_shape and np.array_equal(cwi, wi) and \
                np.array_equal(cwo, wo):
            return dev
    wiT_parts, woT_parts = [], []
    for p in range(N_PAIRS):
        ea, eb = slot_a[p], slot_b[p]
        wiTa = np.ascontiguousarray(wi[ea].transpose(1, 0))  # [D_MODEL, D_FF]
        wiTb = np.ascontiguousarray(wi[eb].transpose(1, 0))
        woTa = np.ascontiguousarray(wo[ea].transpose(1, 0))  # [D_FF, D_MODEL]
        woTb = np.ascontiguousarray(wo[eb].transpose(1, 0))
        for hf in range(2):
            fs = slice(hf * FH, (hf + 1) * FH)
            wiT_parts.append(to_bf16(
                np.concatenate([wiTa[:, fs], wiTb[:, fs]], axis=1)))
            woT_parts.append(to_bf16(
                np.concatenate([woTa[fs, :], woTb[fs, :]], axis=0)))
    dev = {"wiT": jax.device_put(np.concatenate(wiT_parts, axis=0)),
           "woT": jax.device_put(np.concatenate(woT_parts, axis=0))}
    jax.block_until_ready(list(dev.values()))
    _WEIGHT_CACHE.append((wi.copy(), wo.copy(), key_shape, dev))
    del _WEIGHT_CACHE[:-2]
    return dev


def kernel(hidden_states, selected_experts, routing_weights, wi, wo):
    hidden_states = np.asarray(hidden_states)
    selected_experts = np.asarray(selected_experts)
    routing_weights = np.asarray(routing_weights)
    wi = np.asarray(wi)
    wo = np.asarray(wo)

    w_te, idx, slot_a, slot_b, CA, CB = _route(
        hidden_states, selected_experts, routing_weights)
    runner = _get_runner(CA, CB)
    wdev = _pack_weights(wi, wo, slot_a, slot_b, (CA, CB))

    xT_pairs = [pack_x(hidden_states, idx[slot_a[p]], idx[slot_b[p]],
                       CA, CB) for p in range(N_PAIRS)]
    xT = np.concatenate([xT_pairs[c // 2] for c in range(N_CORES)], axis=0)
    concat_in = [{"xT": xT, "wiT": wdev["wiT"], "woT": wdev["woT"]}[name]
                 for name in runner.in_names]

    out_arrs = runner.run_prepped(concat_in)
    C = CA + CB
    yT_all = np.asarray(out_arrs[0]).astype(np.float32).reshape(
        N_CORES, D_MODEL, C)

    out = np.zeros((T, D_MODEL), dtype=np.float32)
    for p in range(N_PAIRS):
        y = yT_all[2 * p] + yT_all[2 * p + 1]            # [D_MODEL, C]
        for seg, (e, c0, cap) in enumerate(
                ((slot_a[p], 0, CA), (slot_b[p], CA, CB))):
            ie = idx[e]
            out[ie] += w_te[ie, e:e + 1] * y[:, c0:c0 + len(ie)].T
    return out
